# revision 1
# baseline (speedup 1.0000x reference)
"""Trainium2 Bass kernel for the CSMAdapter module.

Contract: kernel(**inputs) takes the FULL unsharded inputs (as produced by
the reference setup_inputs()) and returns the FULL output [4, 100, 1024].

Strategy
--------
All weight-only computation is folded on the host (it is data-independent):
    w_proj   = W_in @ Wd.T + bd
    w_prime  = P.T @ w_proj @ P
    masked_w = w_prime * sigmoid(spectral_mask)
    A        = P @ masked_w.T @ P.T          # fused = x @ A
    W_big    = W_in.T @ A                    # fused = llama @ W_big + b_in @ A
The final LayerNorm + mel projection algebra is folded into the mel GEMM:
    mel[m,t] = rstd[t]*(Wg @ h2)[m,t] - (mu[t]*rstd[t])*c1[m] + c2[m]
with Wg = Wmel * ln_g, c1 = Wmel @ ln_g, c2 = Wmel @ ln_b + bmel.

Device (SPMD over 8 cores, data-parallel over the 4096 tokens, 512 each +
2-token conv halos):
    fused_ext = llama_ext @ W_big + b_big (x) vmask     (one K=3072 GEMM)
    conv1 -> gelu -> conv2 as block-diagonal per-tap matmuls (groups=16)
    channel mean / mean-square via ones-vector matmuls
    mel GEMM + rank-1 correction matmuls
Matmuls run in float32r (full fp32 data, fast PE mode).
"""

import sys

import numpy as np


def _ensure_concourse():
    try:
        import concourse  # noqa: F401
    except ImportError:  # pragma: no cover
        for p in ("/opt/trn_rl_repo", "/root/.axon_site/_ro/trn_rl_repo"):
            if p not in sys.path:
                sys.path.insert(0, p)


# ---- static shapes ----
B, T, L, D = 4, 1024, 3072, 1024
NCORES = 8
TOK = 512            # owned tokens per core
EXT = TOK + 4        # fused ext window: tokens -2 .. TOK+2
G1E = TOK + 2        # conv1 ext output: tokens -1 .. TOK+1
KT = L // 128        # 24
DT = D // 128        # 8
NMEL = 100
HB = EXT // 2        # 258  big-GEMM halves
H1 = G1E // 2        # 257  conv1 halves
H2 = TOK // 2        # 256  conv2 halves
GS = 64              # group size (1024 / 16 groups)
GROUPS_ = 16

OFF_BB = 0
OFF_VM = OFF_BB + D
OFF_C1 = OFF_VM + EXT
OFF_C2 = OFF_C1 + NMEL
OFF_EPS = OFF_C2 + NMEL
OFF_ONES = OFF_EPS + 1
SM_LEN = OFF_ONES + TOK

LN_EPS = 1e-5

_PROGRAM = None          # cached (nc, input names)
LAST_RESULTS = None      # BassKernelResults of the most recent run (for test.py)


def _build_program():
    _ensure_concourse()
    from concourse import bacc, tile
    import concourse.mybir as mybir

    f32 = mybir.dt.float32
    f32r = mybir.dt.float32r
    AF = mybir.ActivationFunctionType
    MUL = mybir.AluOpType.mult
    ADD = mybir.AluOpType.add

    nc = bacc.Bacc("TRN2", debug=False, target_bir_lowering=False)

    # DRAM layouts are partition-major so every DMA is contiguous.
    xt_d = nc.dram_tensor("xt", [4, 128, 6, EXT], f32r, kind="ExternalInput")
    wbig_d = nc.dram_tensor("wbig", [DT * 2, 128, KT // 2, 128], f32r,
                            kind="ExternalInput")
    cw1_d = nc.dram_tensor("cw1", [128, DT, 3, 128], f32r, kind="ExternalInput")
    cw2_d = nc.dram_tensor("cw2", [128, DT, 3, 128], f32r, kind="ExternalInput")
    wgt_d = nc.dram_tensor("wgt", [128, DT, NMEL], f32r, kind="ExternalInput")
    cb_d = nc.dram_tensor("cb", [128, 27], f32, kind="ExternalInput")
    sm_d = nc.dram_tensor("smalls", [1, SM_LEN], f32r, kind="ExternalInput")
    onec_d = nc.dram_tensor("onec", [128, 1], f32r, kind="ExternalInput")
    # host-computed halo columns: per d-tile, 4 fused halo cols + 2 g halo cols
    halo_d = nc.dram_tensor("halo", [128, DT, 6], f32r, kind="ExternalInput")
    mel_d = nc.dram_tensor("mel", [NMEL, TOK], f32, kind="ExternalOutput")

    KH = KT // 2

    with tile.TileContext(nc) as tc:
        with (
            tc.tile_pool(name="consts", bufs=1) as consts,
            tc.tile_pool(name="wpool", bufs=5) as wpool,
            tc.tile_pool(name="acts", bufs=1) as acts,
            tc.tile_pool(name="stats", bufs=1) as stats,
            tc.tile_pool(name="ps_mm", bufs=5, space="PSUM") as ps_mm,
            tc.tile_pool(name="ps_st", bufs=2, space="PSUM") as ps_st,
            tc.tile_pool(name="ps_mel", bufs=1, space="PSUM") as ps_mel,
        ):
            # ---- warmup tile via memset (no DMA dependency) ----
            wu_sb = consts.tile([128, 128], f32, name="wu_sb")
            nc.vector.memset(wu_sb, 0.0)
            sm_sb = consts.tile([1, SM_LEN], f32r, name="sm_sb")
            nc.sync.dma_start(out=sm_sb, in_=sm_d[:])
            cb_sb = consts.tile([128, 27], f32, name="cb_sb")
            nc.sync.dma_start(out=cb_sb, in_=cb_d[:])
            ones_col = consts.tile([128, 1], f32r, name="ones_col")
            nc.sync.dma_start(out=ones_col, in_=onec_d[:])
            halo_sb = consts.tile([128, DT, 6], f32r, name="halo_sb")
            nc.sync.dma_start(out=halo_sb, in_=halo_d[:])
            ones_row = sm_sb[0:1, OFF_ONES : OFF_ONES + TOK]

            # ---- PE warmup while input DMAs stream ----
            ps_wu = ps_mm.tile([128, 128], f32, name="ps_wu", tag="mm")
            wu_r = wu_sb.bitcast(f32r)
            for i in range(40):
                nc.tensor.matmul(
                    ps_wu, lhsT=wu_r, rhs=wu_r,
                    start=(i == 0), stop=(i == 39),
                )

            # ---- streaming DMAs ----
            wbh = {}

            def load_wbh(i):
                t = wpool.tile([128, KH, 128], f32r, name=f"wbh{i}", tag="wb")
                nc.sync.dma_start(out=t, in_=wbig_d[i])
                wbh[i] = t

            xg = []

            def load_xg(j):
                t = consts.tile([128, 6, EXT], f32r, name=f"xg{j}", tag=f"xg{j}")
                nc.sync.dma_start(out=t, in_=xt_d[j])
                xg.append(t)

            load_wbh(0)
            load_xg(0)
            load_xg(1)
            load_wbh(1)
            load_xg(2)
            load_xg(3)
            load_wbh(2)
            load_wbh(3)

            cw1_sb = consts.tile([128, DT, 3, 128], f32r, name="cw1_sb")
            cw2_sb = consts.tile([128, DT, 3, 128], f32r, name="cw2_sb")
            wgt_sb = consts.tile([128, DT, NMEL], f32r, name="wgt_sb")

            def xk(k):
                return xg[k // 6][:, k % 6, :]

            fused = [None] * DT
            g = [None] * DT
            h2 = [None] * DT
            h2sq = [None] * DT
            ps_mu_ref = [None]
            ps_sq_ref = [None]
            ps_m_ref = [None]

            def gemm(d):
                fu = acts.tile([128, EXT], f32r, name=f"fu{d}", tag=f"fu{d}")
                fused[d] = fu
                ps = ps_mm.tile([128, TOK], f32, name=f"psA{d}", tag="mm")
                for k in range(KT):
                    if k == 0 and 2 * d + 4 < 2 * DT:
                        load_wbh(2 * d + 4)
                    nc.tensor.matmul(
                        ps,
                        lhsT=wbh[2 * d + k // KH][:, k % KH, :],
                        rhs=xk(k)[:, 2 : 2 + TOK],
                        start=(k == 0), stop=False,
                    )
                nc.tensor.matmul(
                    ps,
                    lhsT=sm_sb[0:1, OFF_BB + d * 128 : OFF_BB + (d + 1) * 128],
                    rhs=ones_row,
                    start=False, stop=True,
                )
                nc.scalar.copy(out=fu[:, 2 : 2 + TOK], in_=ps)
                nc.vector.tensor_copy(fu[:, 0:2], halo_sb[:, d, 0:2])
                nc.vector.tensor_copy(fu[:, EXT - 2 : EXT], halo_sb[:, d, 2:4])
                if 2 * d + 5 < 2 * DT:
                    load_wbh(2 * d + 5)

            def conv1(d):
                # device computes g_ext cols [1, 513); cols 0 and 513 from host
                gd = acts.tile([128, G1E], f32r, name=f"g{d}", tag=f"g{d}")
                g[d] = gd
                ps = ps_mm.tile([128, TOK], f32, name=f"psB{d}", tag="mm")
                for tap in range(3):
                    nc.tensor.matmul(
                        ps, lhsT=cw1_sb[:, d, tap, :],
                        rhs=fused[d][:, 1 + tap : 1 + tap + TOK],
                        start=(tap == 0), stop=(tap == 2),
                    )
                # exact gelu: (conv1+b1) * (0.5 + 0.5*erf((conv1+b1)/sqrt2))
                e = acts.tile([128, TOK], f32, name=f"e{d}", tag="e", bufs=2)
                nc.scalar.activation(
                    out=e, in_=ps, func=AF.Erf,
                    bias=cb_sb[:, d : d + 1], scale=0.7071067811865476,
                )
                h1b = acts.tile([128, TOK], f32, name=f"h1b{d}", tag="h1b",
                                bufs=2)
                nc.vector.tensor_scalar_add(h1b, ps, cb_sb[:, 19 + d : 20 + d])
                nc.vector.tensor_scalar(e, e, 0.5, 0.5, op0=MUL, op1=ADD)
                nc.vector.tensor_mul(gd[:, 1 : 1 + TOK], e, h1b)
                nc.vector.tensor_copy(gd[:, 0:1], halo_sb[:, d, 4:5])
                nc.vector.tensor_copy(gd[:, G1E - 1 : G1E], halo_sb[:, d, 5:6])

            def conv2(d):
                h2d = acts.tile([128, TOK], f32r, name=f"h2{d}", tag=f"h2{d}")
                h2sqd = acts.tile([128, TOK], f32r, name=f"h2sq{d}", tag="h2sq",
                                  bufs=2)
                h2[d] = h2d
                h2sq[d] = h2sqd
                ps = ps_mm.tile([128, TOK], f32, name=f"psC{d}", tag="mm")
                for tap in range(3):
                    nc.tensor.matmul(
                        ps, lhsT=cw2_sb[:, d, tap, :],
                        rhs=g[d][:, tap : tap + TOK],
                        start=(tap == 0), stop=(tap == 2),
                    )
                nc.scalar.add(out=h2d, in_=ps, add=cb_sb[:, 8 + d : 9 + d])
                nc.scalar.activation(
                    out=h2sqd, in_=ps, func=AF.Square,
                    bias=cb_sb[:, 8 + d : 9 + d], scale=1.0,
                )

            def statmm(d):
                if d == 0:
                    ps_mu_ref[0] = ps_st.tile([1, TOK], f32, name="ps_mu",
                                              tag="st")
                    ps_sq_ref[0] = ps_st.tile([1, TOK], f32, name="ps_sq",
                                              tag="st")
                    ps_m_ref[0] = ps_mel.tile([NMEL, TOK], f32, name="ps_m",
                                              tag="mel")
                last = d == DT - 1
                nc.tensor.matmul(ps_mu_ref[0], lhsT=ones_col, rhs=h2[d][:],
                                 start=(d == 0), stop=last)
                nc.tensor.matmul(ps_sq_ref[0], lhsT=ones_col, rhs=h2sq[d][:],
                                 start=(d == 0), stop=last)
                nc.tensor.matmul(ps_m_ref[0], lhsT=wgt_sb[:, d, :], rhs=h2[d][:],
                                 start=(d == 0), stop=last)

            # software-pipelined emission
            for d in range(DT):
                gemm(d)
                if d == 0:
                    nc.scalar.dma_start(out=cw1_sb, in_=cw1_d[:])
                if d == 1:
                    nc.scalar.dma_start(out=cw2_sb, in_=cw2_d[:])
                    nc.scalar.dma_start(out=wgt_sb, in_=wgt_d[:])
                if d >= 1:
                    conv1(d - 1)
                if d >= 2:
                    conv2(d - 2)
                if d >= 3:
                    statmm(d - 3)
            conv1(DT - 1)
            for d in range(DT - 2, DT):
                conv2(d)
                statmm(d - 1)
            statmm(DT - 1)

            # ---- stats on [1, TOK] ----
            ps_mu = ps_mu_ref[0]
            ps_sq = ps_sq_ref[0]
            ps_m = ps_m_ref[0]
            mean = stats.tile([1, TOK], f32, name="mean")
            nc.vector.tensor_scalar_mul(mean, ps_mu, 1.0 / D)
            msq = stats.tile([1, TOK], f32, name="msq", tag="sv", bufs=3)
            nc.vector.tensor_mul(msq, mean, mean)
            var = stats.tile([1, TOK], f32, name="var", tag="sv", bufs=3)
            nc.vector.scalar_tensor_tensor(
                var, in0=ps_sq, scalar=1.0 / D, in1=msq,
                op0=MUL, op1=mybir.AluOpType.subtract,
            )
            sqv = stats.tile([1, TOK], f32, name="sqv", tag="sv", bufs=3)
            nc.scalar.activation(sqv, var, AF.Sqrt,
                                 bias=cb_sb[0:1, 18:19], scale=1.0)
            rstd32 = stats.tile([1, TOK], f32, name="rstd32", tag="sv", bufs=3)
            rscr = stats.tile([1, TOK], f32, name="rscr", tag="sv", bufs=3)
            nc.vector.reciprocal_approx_accurate(rstd32, sqv, scratch=rscr)
            rstd = stats.tile([1, TOK], f32r, name="rstd")
            nc.vector.tensor_copy(rstd, rstd32)
            negu = stats.tile([1, TOK], f32r, name="negu")
            nc.vector.scalar_tensor_tensor(
                negu, in0=mean, scalar=-1.0, in1=rstd32, op0=MUL, op1=MUL,
            )

            # ---- rank-1 corrections + output ----
            ps_r = ps_mm.tile([NMEL, TOK], f32, name="ps_r", tag="mm")
            nc.tensor.matmul(
                ps_r, lhsT=sm_sb[0:1, OFF_C2 : OFF_C2 + NMEL],
                rhs=ones_row, start=True, stop=False,
            )
            nc.tensor.matmul(
                ps_r, lhsT=sm_sb[0:1, OFF_C1 : OFF_C1 + NMEL],
                rhs=negu[0:1, :], start=False, stop=True,
            )
            ps_s = ps_mm.tile([NMEL, TOK], f32, name="ps_s", tag="mm")
            nc.tensor.matmul(
                ps_s, lhsT=sm_sb[0:1, OFF_ONES : OFF_ONES + NMEL],
                rhs=rstd[0:1, :], start=True, stop=True,
            )
            s_sb = stats.tile([NMEL, TOK], f32, name="s_sb")
            nc.vector.tensor_copy(s_sb, ps_s)
            out_sb = stats.tile([NMEL, TOK], f32, name="out_sb")
            nc.vector.tensor_mul(out_sb, ps_m, s_sb)
            nc.vector.tensor_add(out_sb, out_sb, ps_r)
            nc.sync.dma_start(out=mel_d[:], in_=out_sb)

    nc.compile()
    return nc


def _sigmoid64(x):
    return 1.0 / (1.0 + np.exp(-x.astype(np.float64)))


def host_prep(inputs):
    """Fold all data-independent computation; build per-core device inputs.

    Returns (shared, per_core) where shared is a dict of replicated arrays
    and per_core is a list of 8 dicts with the core-specific arrays.
    """
    f32 = np.float32
    W_in = np.asarray(inputs["W_in"], dtype=np.float64)
    Wd = np.asarray(inputs["Wd"], dtype=np.float64)
    bd = np.asarray(inputs["bd"], dtype=np.float64)
    P = np.asarray(inputs["P"], dtype=np.float64)
    smask = np.asarray(inputs["spectral_mask"], dtype=np.float64)
    b_in = np.asarray(inputs["b_in"], dtype=np.float64)

    w_proj = W_in @ Wd.T + bd[None, :]
    w_prime = P.T @ w_proj @ P
    masked_w = w_prime * _sigmoid64(smask)
    A = P @ masked_w.T @ P.T
    W_big64 = W_in.T @ A                                       # [L, D] f64
    b_big64 = b_in @ A                                         # [D] f64
    W_big = np.ascontiguousarray(W_big64, dtype=f32)
    b_big = b_big64.astype(f32)

    # [2d+half, kp, k_in_half, dc] (partition-major, half k-slices)
    wbig_t = np.ascontiguousarray(
        W_big.reshape(2, KT // 2, 128, DT, 128).transpose(3, 0, 2, 1, 4)
    ).reshape(DT * 2, 128, KT // 2, 128)

    def blockdiag(w):
        w = np.asarray(w, dtype=f32)  # [C, GS, 3]
        out = np.zeros((DT, 3, 128, 128), dtype=f32)
        for d in range(DT):
            for co in range(128):
                c = d * 128 + co
                blk = co // GS
                # out[d, tap, blk*GS + i, co] = w[c, i, tap]
                out[d, :, blk * GS : (blk + 1) * GS, co] = w[c].T
        return out

    cw1_t = np.ascontiguousarray(blockdiag(inputs["conv1_w"]).transpose(2, 0, 1, 3))
    cw2_t = np.ascontiguousarray(blockdiag(inputs["conv2_w"]).transpose(2, 0, 1, 3))

    Wmel = np.asarray(inputs["Wmel"], dtype=np.float64)
    ln_g = np.asarray(inputs["ln_g"], dtype=np.float64)
    ln_b = np.asarray(inputs["ln_b"], dtype=np.float64)
    bmel = np.asarray(inputs["bmel"], dtype=np.float64)
    Wg = (Wmel * ln_g[None, :]).astype(f32)                    # [NMEL, D]
    wgt_t = np.ascontiguousarray(
        Wg.T.reshape(DT, 128, NMEL).transpose(1, 0, 2)
    )  # [kp, d, m]
    c1 = (Wmel @ ln_g).astype(f32)
    c2 = (Wmel @ ln_b + bmel).astype(f32)

    cb_base = np.zeros((128, 27), dtype=f32)
    cb_base[:, 18] = LN_EPS
    b1_cols = np.asarray(inputs["conv1_b"], dtype=f32).reshape(DT, 128).T
    cb_base[:, 0:8] = b1_cols * np.float32(0.7071067811865476)  # pre-scaled for Erf
    cb_base[:, 8:16] = np.asarray(inputs["conv2_b"], dtype=f32).reshape(DT, 128).T
    cb_base[:, 19:27] = b1_cols

    llama = np.asarray(inputs["llama_embeddings"], dtype=f32).reshape(B * T, L)
    conv1_w_np = np.asarray(inputs["conv1_w"], dtype=np.float64)  # [D, GS, 3]
    conv1_b_np = np.asarray(inputs["conv1_b"], dtype=np.float64)
    gidx = np.arange(D) // GS

    import math
    _erf_vec = np.vectorize(math.erf)

    def _gelu64(x):
        return x * 0.5 * (1.0 + _erf_vec(x / math.sqrt(2.0)))

    shared = dict(wbig=wbig_t, cw1=cw1_t, cw2=cw2_t, wgt=wgt_t,
                  onec=np.ones((128, 1), dtype=f32))
    per_core = []
    for c in range(NCORES):
        b, h = divmod(c, 2)
        start = b * T + h * TOK
        ext_idx = np.arange(start - 2, start + TOK + 2)
        valid = (ext_idx >= b * T) & (ext_idx < (b + 1) * T)
        xext = np.zeros((EXT, L), dtype=f32)
        xext[valid] = llama[ext_idx[valid]]
        xt = np.ascontiguousarray(
            xext.T.reshape(4, 6, 128, EXT).transpose(0, 2, 1, 3)
        )  # [j, p, kk, t]

        # host-computed halo columns (exact fp32-grade)
        def fcol(u):
            gu = start + u
            if b * T <= gu < (b + 1) * T:
                return llama[gu].astype(np.float64) @ W_big64 + b_big64
            return np.zeros(D, dtype=np.float64)

        def conv1col(m3):
            # m3: [D, 3] inputs for taps 0..2 -> conv1 + bias, gelu
            in_g = m3.reshape(GROUPS_, GS, 3)[gidx]       # [D, GS, 3]
            out = np.einsum("cit,cit->c", conv1_w_np, in_g) + conv1_b_np
            return _gelu64(out)

        fm2, fm1, f0 = fcol(-2), fcol(-1), fcol(0)
        f510, f511 = fcol(510), fcol(511)
        f512, f513 = fcol(TOK), fcol(TOK + 1)
        if h == 1:
            g_left = conv1col(np.stack([fm2, fm1, f0], axis=1))
        else:
            g_left = np.zeros(D, dtype=np.float64)
        if h == 0:
            g_right = conv1col(np.stack([f511, f512, f513], axis=1))
        else:
            g_right = np.zeros(D, dtype=np.float64)
        halo = np.zeros((128, DT, 6), dtype=f32)
        for dd in range(DT):
            slc = slice(dd * 128, (dd + 1) * 128)
            halo[:, dd, 0] = fm2[slc]
            halo[:, dd, 1] = fm1[slc]
            halo[:, dd, 2] = f512[slc]
            halo[:, dd, 3] = f513[slc]
            halo[:, dd, 4] = g_left[slc]
            halo[:, dd, 5] = g_right[slc]

        sm = np.zeros((1, SM_LEN), dtype=f32)
        sm[0, OFF_BB : OFF_BB + D] = b_big
        sm[0, OFF_VM : OFF_VM + EXT] = valid.astype(f32)
        sm[0, OFF_C1 : OFF_C1 + NMEL] = c1
        sm[0, OFF_C2 : OFF_C2 + NMEL] = c2
        sm[0, OFF_EPS] = LN_EPS
        sm[0, OFF_ONES : OFF_ONES + TOK] = 1.0

        cb = cb_base.copy()
        # g halo validity: col 16 -> token -1, col 17 -> token TOK
        cb[:, 16] = 1.0 if h == 1 else 0.0
        cb[:, 17] = 1.0 if h == 0 else 0.0

        per_core.append(dict(xt=xt, smalls=sm, cb=cb, halo=halo))
    return shared, per_core


def _ensure_axon_hooks():
    """If this image's antenv lacks axon_hooks (needed by bass_utils when
    BASS_TRACE is set under axon), register a functional ctypes-based hook so
    tracing degrades gracefully instead of crashing."""
    try:
        import antenv.axon_hooks  # noqa: F401
        return
    except ImportError:
        pass
    try:
        import contextlib
        import ctypes
        import types

        hook = None
        try:
            lib = ctypes.CDLL("/opt/axon/libaxon_pjrt.so")
            if hasattr(lib, "axon_start_nrt_profile"):
                lib.axon_start_nrt_profile.argtypes = [
                    ctypes.POINTER(ctypes.c_int64),
                    ctypes.c_size_t,
                ]
                lib.axon_start_nrt_profile.restype = ctypes.c_int64
                lib.axon_stop_nrt_profile.argtypes = [ctypes.c_char_p]
                lib.axon_stop_nrt_profile.restype = ctypes.c_int64

                @contextlib.contextmanager
                def hook(output_dir, device_ids):
                    import jax

                    jax.devices()
                    if device_ids:
                        ids = (ctypes.c_int64 * len(device_ids))(*device_ids)
                        rc = lib.axon_start_nrt_profile(ids, len(device_ids))
                    else:
                        rc = lib.axon_start_nrt_profile(None, 0)
                    if rc != 0:
                        raise RuntimeError(f"axon_start_nrt_profile rc={rc}")
                    try:
                        yield
                    finally:
                        lib.axon_stop_nrt_profile(str(output_dir).encode())
        except OSError:
            hook = None

        mod = types.ModuleType("antenv.axon_hooks")
        mod.get_axon_ntff_profile_hook = lambda: hook
        mod.set_axon_ntff_profile_hook = lambda h: None
        sys.modules["antenv.axon_hooks"] = mod
        import antenv

        antenv.axon_hooks = mod
    except Exception:
        pass


def kernel(**inputs):
    global _PROGRAM, LAST_RESULTS
    _ensure_concourse()
    _ensure_axon_hooks()
    from concourse import bass_utils

    if _PROGRAM is None:
        _PROGRAM = _build_program()
    nc = _PROGRAM

    shared, per_core = host_prep(inputs)
    in_maps = [{**shared, **pc} for pc in per_core]

    res = None
    last_exc = None
    for _attempt in range(3):
        try:
            res = bass_utils.run_bass_kernel_spmd(
                nc, in_maps, core_ids=list(range(NCORES))
            )
            break
        except Exception as exc:  # transient NRT device errors happen
            last_exc = exc
    if res is None:
        raise last_exc
    LAST_RESULTS = res

    out = np.zeros((B, NMEL, T), dtype=np.float32)
    for c in range(NCORES):
        b, h = divmod(c, 2)
        out[b, :, h * TOK : (h + 1) * TOK] = res.results[c]["mel"]
    return out



# revision 13
# speedup vs baseline: 1.2962x; 1.2962x over previous
"""Trainium2 Bass kernel for the CSMAdapter module.

Contract: kernel(**inputs) takes the FULL unsharded inputs (as produced by
the reference setup_inputs()) and returns the FULL output [4, 100, 1024].

Strategy
--------
All weight-only computation is folded on the host (it is data-independent):
    w_proj   = W_in @ Wd.T + bd
    w_prime  = P.T @ w_proj @ P
    masked_w = w_prime * sigmoid(spectral_mask)
    A        = P @ masked_w.T @ P.T          # fused = x @ A
    W_big    = W_in.T @ A                    # fused = llama @ W_big + b_in @ A
The final LayerNorm + mel projection algebra is folded into the mel GEMM:
    mel[m,t] = rstd[t]*(Wg @ h2)[m,t] - (mu[t]*rstd[t])*c1[m] + c2[m]
with Wg = Wmel * ln_g, c1 = Wmel @ ln_g, c2 = Wmel @ ln_b + bmel.

Device (SPMD over 8 cores, data-parallel over the 4096 tokens, 512 each +
2-token conv halos):
    fused_ext = llama_ext @ W_big + b_big (x) vmask     (one K=3072 GEMM)
    conv1 -> gelu -> conv2 as block-diagonal per-tap matmuls (groups=16)
    channel mean / mean-square via ones-vector matmuls
    mel GEMM + rank-1 correction matmuls
Matmuls run in float32r (full fp32 data, fast PE mode).
"""

import sys

import numpy as np


def _ensure_concourse():
    try:
        import concourse  # noqa: F401
    except ImportError:  # pragma: no cover
        for p in ("/opt/trn_rl_repo", "/root/.axon_site/_ro/trn_rl_repo"):
            if p not in sys.path:
                sys.path.insert(0, p)


# ---- static shapes ----
B, T, L, D = 4, 1024, 3072, 1024
NCORES = 8
TOK = 512            # owned tokens per core
EXT = TOK + 4        # fused ext window: tokens -2 .. TOK+2
G1E = TOK + 2        # conv1 ext output: tokens -1 .. TOK+1
KT = L // 128        # 24
DT = D // 128        # 8
NMEL = 100
HB = EXT // 2        # 258  big-GEMM halves
H1 = G1E // 2        # 257  conv1 halves
H2 = TOK // 2        # 256  conv2 halves
GS = 64              # group size (1024 / 16 groups)
GROUPS_ = 16

OFF_BB = 0
OFF_VM = OFF_BB + D
OFF_C1 = OFF_VM + EXT
OFF_C2 = OFF_C1 + NMEL
OFF_EPS = OFF_C2 + NMEL
OFF_ONES = OFF_EPS + 1
SM_LEN = OFF_ONES + TOK

LN_EPS = 1e-5

_PROGRAM = None          # cached (nc, input names)
LAST_RESULTS = None      # BassKernelResults of the most recent run (for test.py)


def _build_program():
    _ensure_concourse()
    from concourse import bacc, tile
    import concourse.mybir as mybir

    f32 = mybir.dt.float32
    f32r = mybir.dt.float32r
    bf16 = mybir.dt.bfloat16
    AF = mybir.ActivationFunctionType
    MUL = mybir.AluOpType.mult
    ADD = mybir.AluOpType.add

    nc = bacc.Bacc("TRN2", debug=False, target_bir_lowering=False)

    # DRAM layouts are partition-major so every DMA is contiguous.
    xt_d = nc.dram_tensor("xt", [4, 128, 6, EXT], bf16, kind="ExternalInput")
    wbig_d = nc.dram_tensor("wbig", [DT * 2, 128, KT // 2, 128], bf16,
                            kind="ExternalInput")
    cw1_d = nc.dram_tensor("cw1", [128, DT, 3, 128], bf16, kind="ExternalInput")
    cw2_d = nc.dram_tensor("cw2", [128, DT, 3, 128], bf16, kind="ExternalInput")
    wgt_d = nc.dram_tensor("wgt", [128, DT, NMEL], f32r, kind="ExternalInput")
    cb_d = nc.dram_tensor("cb", [128, 35], f32, kind="ExternalInput")
    sm_d = nc.dram_tensor("smalls", [1, SM_LEN], f32r, kind="ExternalInput")
    onec_d = nc.dram_tensor("onec", [128, 1], f32r, kind="ExternalInput")
    # host-computed halo columns: per d-tile, 4 fused halo cols + 2 g halo cols
    halo_d = nc.dram_tensor("halo", [128, DT, 6], bf16, kind="ExternalInput")
    mel_d = nc.dram_tensor("mel", [NMEL, TOK], f32, kind="ExternalOutput")

    KH = KT // 2

    with tile.TileContext(nc) as tc:
        with (
            tc.tile_pool(name="consts", bufs=1) as consts,
            tc.tile_pool(name="wpool", bufs=5) as wpool,
            tc.tile_pool(name="acts", bufs=1) as acts,
            tc.tile_pool(name="stats", bufs=1) as stats,
            tc.tile_pool(name="ps_mm", bufs=5, space="PSUM") as ps_mm,
            tc.tile_pool(name="ps_st", bufs=2, space="PSUM") as ps_st,
            tc.tile_pool(name="ps_mel", bufs=1, space="PSUM") as ps_mel,
        ):
            # ---- warmup tile via memset (no DMA dependency) ----
            wu_sb = consts.tile([128, 128], f32, name="wu_sb")
            nc.vector.memset(wu_sb, 0.0)
            sm_sb = consts.tile([1, SM_LEN], f32r, name="sm_sb")
            nc.sync.dma_start(out=sm_sb, in_=sm_d[:])
            cb_sb = consts.tile([128, 35], f32, name="cb_sb")
            nc.sync.dma_start(out=cb_sb, in_=cb_d[:])
            ones_col = consts.tile([128, 1], f32r, name="ones_col")
            nc.sync.dma_start(out=ones_col, in_=onec_d[:])
            halo_sb = consts.tile([128, DT, 6], bf16, name="halo_sb")
            nc.sync.dma_start(out=halo_sb, in_=halo_d[:])
            ones_row = sm_sb[0:1, OFF_ONES : OFF_ONES + TOK]

            # ---- PE warmup while input DMAs stream ----
            ps_wu = ps_mm.tile([128, 128], f32, name="ps_wu", tag="mm")
            wu_r = wu_sb.bitcast(f32r)
            for i in range(40):
                nc.tensor.matmul(
                    ps_wu, lhsT=wu_r, rhs=wu_r,
                    start=(i == 0), stop=(i == 39),
                )

            # ---- streaming DMAs ----
            wbh = {}

            def load_wbh(i):
                t = wpool.tile([128, KH, 128], bf16, name=f"wbh{i}", tag="wb")
                nc.sync.dma_start(out=t, in_=wbig_d[i])
                wbh[i] = t

            xg = []

            def load_xg(j):
                t = consts.tile([128, 6, EXT], bf16, name=f"xg{j}", tag=f"xg{j}")
                nc.sync.dma_start(out=t, in_=xt_d[j])
                xg.append(t)

            load_wbh(0)
            load_xg(0)
            load_xg(1)
            load_wbh(1)
            load_xg(2)
            load_xg(3)
            load_wbh(2)
            load_wbh(3)

            cw1_sb = consts.tile([128, DT, 3, 128], bf16, name="cw1_sb")
            cw2_sb = consts.tile([128, DT, 3, 128], bf16, name="cw2_sb")
            wgt_sb = consts.tile([128, DT, NMEL], f32r, name="wgt_sb")

            def xk(k):
                return xg[k // 6][:, k % 6, :]

            fused = [None] * DT
            g = [None] * DT
            h2 = [None] * DT
            h2sq = [None] * DT
            ps_mu_ref = [None]
            ps_sq_ref = [None]
            ps_m_ref = [None]

            def gemm(d):
                fu = acts.tile([128, EXT], bf16, name=f"fu{d}", tag=f"fu{d}")
                fused[d] = fu
                ps = ps_mm.tile([128, TOK], f32, name=f"psA{d}", tag="mm")
                for k in range(KT):
                    if k == 0 and 2 * d + 4 < 2 * DT:
                        load_wbh(2 * d + 4)
                    nc.tensor.matmul(
                        ps,
                        lhsT=wbh[2 * d + k // KH][:, k % KH, :],
                        rhs=xk(k)[:, 2 : 2 + TOK],
                        start=(k == 0), stop=(k == KT - 1),
                    )
                # bias b_big folded into the PSUM->SBUF copy (cb cols 27:35)
                nc.scalar.add(out=fu[:, 2 : 2 + TOK], in_=ps,
                              add=cb_sb[:, 27 + d : 28 + d])
                nc.vector.tensor_copy(fu[:, 0:2], halo_sb[:, d, 0:2])
                nc.vector.tensor_copy(fu[:, EXT - 2 : EXT], halo_sb[:, d, 2:4])
                if 2 * d + 5 < 2 * DT:
                    load_wbh(2 * d + 5)

            def conv1(d):
                # device computes g_ext cols [1, 513); cols 0 and 513 from host
                gd = acts.tile([128, G1E], bf16, name=f"g{d}", tag=f"g{d}")
                g[d] = gd
                ps = ps_mm.tile([128, TOK], f32, name=f"psB{d}", tag="mm")
                for tap in range(3):
                    nc.tensor.matmul(
                        ps, lhsT=cw1_sb[:, d, tap, :],
                        rhs=fused[d][:, 1 + tap : 1 + tap + TOK],
                        start=(tap == 0), stop=(tap == 2),
                    )
                # exact gelu: (conv1+b1) * (0.5 + 0.5*erf((conv1+b1)/sqrt2))
                e = acts.tile([128, TOK], f32, name=f"e{d}", tag="e", bufs=2)
                nc.scalar.activation(
                    out=e, in_=ps, func=AF.Erf,
                    bias=cb_sb[:, d : d + 1], scale=0.7071067811865476,
                )
                h1b = acts.tile([128, TOK], f32, name=f"h1b{d}", tag="h1b",
                                bufs=2)
                nc.vector.tensor_scalar_add(h1b, ps, cb_sb[:, 19 + d : 20 + d])
                nc.vector.tensor_scalar(e, e, 0.5, 0.5, op0=MUL, op1=ADD)
                nc.vector.tensor_mul(gd[:, 1 : 1 + TOK], e, h1b)
                nc.vector.tensor_copy(gd[:, 0:1], halo_sb[:, d, 4:5])
                nc.vector.tensor_copy(gd[:, G1E - 1 : G1E], halo_sb[:, d, 5:6])

            def conv2(d):
                h2d = acts.tile([128, TOK], f32r, name=f"h2{d}", tag=f"h2{d}")
                h2sqd = acts.tile([128, TOK], f32r, name=f"h2sq{d}", tag="h2sq",
                                  bufs=2)
                h2[d] = h2d
                h2sq[d] = h2sqd
                ps = ps_mm.tile([128, TOK], f32, name=f"psC{d}", tag="mm")
                for tap in range(3):
                    nc.tensor.matmul(
                        ps, lhsT=cw2_sb[:, d, tap, :],
                        rhs=g[d][:, tap : tap + TOK],
                        start=(tap == 0), stop=(tap == 2),
                    )
                nc.scalar.add(out=h2d, in_=ps, add=cb_sb[:, 8 + d : 9 + d])
                nc.scalar.activation(
                    out=h2sqd, in_=ps, func=AF.Square,
                    bias=cb_sb[:, 8 + d : 9 + d], scale=1.0,
                )

            def statmm(d):
                if d == 0:
                    ps_mu_ref[0] = ps_st.tile([1, TOK], f32, name="ps_mu",
                                              tag="st")
                    ps_sq_ref[0] = ps_st.tile([1, TOK], f32, name="ps_sq",
                                              tag="st")
                    ps_m_ref[0] = ps_mel.tile([NMEL, TOK], f32, name="ps_m",
                                              tag="mel")
                last = d == DT - 1
                nc.tensor.matmul(ps_mu_ref[0], lhsT=ones_col, rhs=h2[d][:],
                                 start=(d == 0), stop=last)
                nc.tensor.matmul(ps_sq_ref[0], lhsT=ones_col, rhs=h2sq[d][:],
                                 start=(d == 0), stop=last)
                nc.tensor.matmul(ps_m_ref[0], lhsT=wgt_sb[:, d, :], rhs=h2[d][:],
                                 start=(d == 0), stop=last)

            # software-pipelined emission
            for d in range(DT):
                gemm(d)
                if d == 0:
                    nc.scalar.dma_start(out=cw1_sb, in_=cw1_d[:])
                if d == 1:
                    nc.scalar.dma_start(out=cw2_sb, in_=cw2_d[:])
                    nc.scalar.dma_start(out=wgt_sb, in_=wgt_d[:])
                if d >= 1:
                    conv1(d - 1)
                if d >= 2:
                    conv2(d - 2)
                if d >= 3:
                    statmm(d - 3)
            conv1(DT - 1)
            for d in range(DT - 2, DT):
                conv2(d)
                statmm(d - 1)
            statmm(DT - 1)

            # ---- stats on [1, TOK] ----
            ps_mu = ps_mu_ref[0]
            ps_sq = ps_sq_ref[0]
            ps_m = ps_m_ref[0]
            mean = stats.tile([1, TOK], f32, name="mean")
            nc.vector.tensor_scalar_mul(mean, ps_mu, 1.0 / D)
            msq = stats.tile([1, TOK], f32, name="msq", tag="sv", bufs=3)
            nc.vector.tensor_mul(msq, mean, mean)
            var = stats.tile([1, TOK], f32, name="var", tag="sv", bufs=3)
            nc.vector.scalar_tensor_tensor(
                var, in0=ps_sq, scalar=1.0 / D, in1=msq,
                op0=MUL, op1=mybir.AluOpType.subtract,
            )
            sqv = stats.tile([1, TOK], f32, name="sqv", tag="sv", bufs=3)
            nc.scalar.activation(sqv, var, AF.Sqrt,
                                 bias=cb_sb[0:1, 18:19], scale=1.0)
            rstd32 = stats.tile([1, TOK], f32, name="rstd32", tag="sv", bufs=3)
            rscr = stats.tile([1, TOK], f32, name="rscr", tag="sv", bufs=3)
            nc.vector.reciprocal_approx_accurate(rstd32, sqv, scratch=rscr)
            rstd = stats.tile([1, TOK], f32r, name="rstd")
            nc.vector.tensor_copy(rstd, rstd32)
            negu = stats.tile([1, TOK], f32r, name="negu")
            nc.vector.scalar_tensor_tensor(
                negu, in0=mean, scalar=-1.0, in1=rstd32, op0=MUL, op1=MUL,
            )

            # ---- rank-1 corrections + output ----
            ps_r = ps_mm.tile([NMEL, TOK], f32, name="ps_r", tag="mm")
            nc.tensor.matmul(
                ps_r, lhsT=sm_sb[0:1, OFF_C2 : OFF_C2 + NMEL],
                rhs=ones_row, start=True, stop=False,
            )
            nc.tensor.matmul(
                ps_r, lhsT=sm_sb[0:1, OFF_C1 : OFF_C1 + NMEL],
                rhs=negu[0:1, :], start=False, stop=True,
            )
            ps_s = ps_mm.tile([NMEL, TOK], f32, name="ps_s", tag="mm")
            nc.tensor.matmul(
                ps_s, lhsT=sm_sb[0:1, OFF_ONES : OFF_ONES + NMEL],
                rhs=rstd[0:1, :], start=True, stop=True,
            )
            s_sb = stats.tile([NMEL, TOK], f32, name="s_sb")
            nc.vector.tensor_copy(s_sb, ps_s)
            out_sb = stats.tile([NMEL, TOK], f32, name="out_sb")
            nc.vector.tensor_mul(out_sb, ps_m, s_sb)
            nc.vector.tensor_add(out_sb, out_sb, ps_r)
            nc.sync.dma_start(out=mel_d[:], in_=out_sb)

    nc.compile()
    return nc


def _sigmoid64(x):
    return 1.0 / (1.0 + np.exp(-x.astype(np.float64)))


def host_prep(inputs):
    """Fold all data-independent computation; build per-core device inputs.

    Returns (shared, per_core) where shared is a dict of replicated arrays
    and per_core is a list of 8 dicts with the core-specific arrays.
    """
    import ml_dtypes

    bf16 = ml_dtypes.bfloat16
    f32 = np.float32
    W_in = np.asarray(inputs["W_in"], dtype=np.float64)
    Wd = np.asarray(inputs["Wd"], dtype=np.float64)
    bd = np.asarray(inputs["bd"], dtype=np.float64)
    P = np.asarray(inputs["P"], dtype=np.float64)
    smask = np.asarray(inputs["spectral_mask"], dtype=np.float64)
    b_in = np.asarray(inputs["b_in"], dtype=np.float64)

    w_proj = W_in @ Wd.T + bd[None, :]
    w_prime = P.T @ w_proj @ P
    masked_w = w_prime * _sigmoid64(smask)
    A = P @ masked_w.T @ P.T
    W_big64 = W_in.T @ A                                       # [L, D] f64
    b_big64 = b_in @ A                                         # [D] f64
    W_big = np.ascontiguousarray(W_big64, dtype=f32)
    b_big = b_big64.astype(f32)

    # [2d+half, kp, k_in_half, dc] (partition-major, half k-slices)
    wbig_t = np.ascontiguousarray(
        W_big.reshape(2, KT // 2, 128, DT, 128).transpose(3, 0, 2, 1, 4)
    ).reshape(DT * 2, 128, KT // 2, 128).astype(bf16)

    def blockdiag(w):
        w = np.asarray(w, dtype=f32)  # [C, GS, 3]
        out = np.zeros((DT, 3, 128, 128), dtype=f32)
        for d in range(DT):
            for co in range(128):
                c = d * 128 + co
                blk = co // GS
                # out[d, tap, blk*GS + i, co] = w[c, i, tap]
                out[d, :, blk * GS : (blk + 1) * GS, co] = w[c].T
        return out

    cw1_t = np.ascontiguousarray(
        blockdiag(inputs["conv1_w"]).transpose(2, 0, 1, 3)
    ).astype(bf16)
    cw2_t = np.ascontiguousarray(
        blockdiag(inputs["conv2_w"]).transpose(2, 0, 1, 3)
    ).astype(bf16)

    Wmel = np.asarray(inputs["Wmel"], dtype=np.float64)
    ln_g = np.asarray(inputs["ln_g"], dtype=np.float64)
    ln_b = np.asarray(inputs["ln_b"], dtype=np.float64)
    bmel = np.asarray(inputs["bmel"], dtype=np.float64)
    Wg = (Wmel * ln_g[None, :]).astype(f32)                    # [NMEL, D]
    wgt_t = np.ascontiguousarray(
        Wg.T.reshape(DT, 128, NMEL).transpose(1, 0, 2)
    )  # [kp, d, m]
    c1 = (Wmel @ ln_g).astype(f32)
    c2 = (Wmel @ ln_b + bmel).astype(f32)

    cb_base = np.zeros((128, 35), dtype=f32)
    cb_base[:, 18] = LN_EPS
    b1_cols = np.asarray(inputs["conv1_b"], dtype=f32).reshape(DT, 128).T
    cb_base[:, 0:8] = b1_cols * np.float32(0.7071067811865476)  # pre-scaled for Erf
    cb_base[:, 8:16] = np.asarray(inputs["conv2_b"], dtype=f32).reshape(DT, 128).T
    cb_base[:, 19:27] = b1_cols
    cb_base[:, 27:35] = b_big.reshape(DT, 128).T

    llama = np.asarray(inputs["llama_embeddings"], dtype=f32).reshape(B * T, L)
    conv1_w_np = np.asarray(inputs["conv1_w"], dtype=np.float64)  # [D, GS, 3]
    conv1_b_np = np.asarray(inputs["conv1_b"], dtype=np.float64)
    gidx = np.arange(D) // GS

    import math
    _erf_vec = np.vectorize(math.erf)

    def _gelu64(x):
        return x * 0.5 * (1.0 + _erf_vec(x / math.sqrt(2.0)))

    shared = dict(wbig=wbig_t, cw1=cw1_t, cw2=cw2_t, wgt=wgt_t,
                  onec=np.ones((128, 1), dtype=f32))
    per_core = []
    for c in range(NCORES):
        b, h = divmod(c, 2)
        start = b * T + h * TOK
        ext_idx = np.arange(start - 2, start + TOK + 2)
        valid = (ext_idx >= b * T) & (ext_idx < (b + 1) * T)
        xext = np.zeros((EXT, L), dtype=f32)
        xext[valid] = llama[ext_idx[valid]]
        xt = np.ascontiguousarray(
            xext.T.reshape(4, 6, 128, EXT).transpose(0, 2, 1, 3)
        ).astype(bf16)  # [j, p, kk, t]

        # host-computed halo columns (exact fp32-grade)
        def fcol(u):
            gu = start + u
            if b * T <= gu < (b + 1) * T:
                return llama[gu].astype(np.float64) @ W_big64 + b_big64
            return np.zeros(D, dtype=np.float64)

        def conv1col(m3):
            # m3: [D, 3] inputs for taps 0..2 -> conv1 + bias, gelu
            in_g = m3.reshape(GROUPS_, GS, 3)[gidx]       # [D, GS, 3]
            out = np.einsum("cit,cit->c", conv1_w_np, in_g) + conv1_b_np
            return _gelu64(out)

        fm2, fm1, f0 = fcol(-2), fcol(-1), fcol(0)
        f510, f511 = fcol(510), fcol(511)
        f512, f513 = fcol(TOK), fcol(TOK + 1)
        if h == 1:
            g_left = conv1col(np.stack([fm2, fm1, f0], axis=1))
        else:
            g_left = np.zeros(D, dtype=np.float64)
        if h == 0:
            g_right = conv1col(np.stack([f511, f512, f513], axis=1))
        else:
            g_right = np.zeros(D, dtype=np.float64)
        halo = np.zeros((128, DT, 6), dtype=bf16)
        for dd in range(DT):
            slc = slice(dd * 128, (dd + 1) * 128)
            halo[:, dd, 0] = fm2[slc]
            halo[:, dd, 1] = fm1[slc]
            halo[:, dd, 2] = f512[slc]
            halo[:, dd, 3] = f513[slc]
            halo[:, dd, 4] = g_left[slc]
            halo[:, dd, 5] = g_right[slc]

        sm = np.zeros((1, SM_LEN), dtype=f32)
        sm[0, OFF_BB : OFF_BB + D] = b_big
        sm[0, OFF_VM : OFF_VM + EXT] = valid.astype(f32)
        sm[0, OFF_C1 : OFF_C1 + NMEL] = c1
        sm[0, OFF_C2 : OFF_C2 + NMEL] = c2
        sm[0, OFF_EPS] = LN_EPS
        sm[0, OFF_ONES : OFF_ONES + TOK] = 1.0

        cb = cb_base.copy()
        # g halo validity: col 16 -> token -1, col 17 -> token TOK
        cb[:, 16] = 1.0 if h == 1 else 0.0
        cb[:, 17] = 1.0 if h == 0 else 0.0

        per_core.append(dict(xt=xt, smalls=sm, cb=cb, halo=halo))
    return shared, per_core


def _ensure_axon_hooks():
    """If this image's antenv lacks axon_hooks (needed by bass_utils when
    BASS_TRACE is set under axon), register a functional ctypes-based hook so
    tracing degrades gracefully instead of crashing."""
    try:
        import antenv.axon_hooks  # noqa: F401
        return
    except ImportError:
        pass
    try:
        import contextlib
        import ctypes
        import types

        hook = None
        try:
            lib = ctypes.CDLL("/opt/axon/libaxon_pjrt.so")
            if hasattr(lib, "axon_start_nrt_profile"):
                lib.axon_start_nrt_profile.argtypes = [
                    ctypes.POINTER(ctypes.c_int64),
                    ctypes.c_size_t,
                ]
                lib.axon_start_nrt_profile.restype = ctypes.c_int64
                lib.axon_stop_nrt_profile.argtypes = [ctypes.c_char_p]
                lib.axon_stop_nrt_profile.restype = ctypes.c_int64

                @contextlib.contextmanager
                def hook(output_dir, device_ids):
                    import jax

                    jax.devices()
                    if device_ids:
                        ids = (ctypes.c_int64 * len(device_ids))(*device_ids)
                        rc = lib.axon_start_nrt_profile(ids, len(device_ids))
                    else:
                        rc = lib.axon_start_nrt_profile(None, 0)
                    if rc != 0:
                        raise RuntimeError(f"axon_start_nrt_profile rc={rc}")
                    try:
                        yield
                    finally:
                        lib.axon_stop_nrt_profile(str(output_dir).encode())
        except OSError:
            hook = None

        mod = types.ModuleType("antenv.axon_hooks")
        mod.get_axon_ntff_profile_hook = lambda: hook
        mod.set_axon_ntff_profile_hook = lambda h: None
        sys.modules["antenv.axon_hooks"] = mod
        import antenv

        antenv.axon_hooks = mod
    except Exception:
        pass


def kernel(**inputs):
    global _PROGRAM, LAST_RESULTS
    _ensure_concourse()
    _ensure_axon_hooks()
    from concourse import bass_utils

    if _PROGRAM is None:
        _PROGRAM = _build_program()
    nc = _PROGRAM

    shared, per_core = host_prep(inputs)
    in_maps = [{**shared, **pc} for pc in per_core]

    res = None
    last_exc = None
    for _attempt in range(3):
        try:
            res = bass_utils.run_bass_kernel_spmd(
                nc, in_maps, core_ids=list(range(NCORES))
            )
            break
        except Exception as exc:  # transient NRT device errors happen
            last_exc = exc
    if res is None:
        raise last_exc
    LAST_RESULTS = res

    out = np.zeros((B, NMEL, T), dtype=np.float32)
    for c in range(NCORES):
        b, h = divmod(c, 2)
        out[b, :, h * TOK : (h + 1) * TOK] = res.results[c]["mel"]
    return out

